# revision 9
# baseline (speedup 1.0000x reference)
"""Trainium2 Bass kernel for LittleBitLinearHF.

Computation (per reference):
    y = ((x * g) @ sign(V) * ell) @ sign(U).T * h + bias
with x (4, 2048, 4096) f32, U/V (4096, 128), rank r=128.

Strategy (memory-roofline oriented; tolerance is rel_err < 2e-2):
  * Data-parallel: 8192 tokens over 8 cores (1024 each), params replicated.
  * Quantization plan (host-side):
      - xq  = e3m4(x * g * 2^-k[d])   1 byte/elt  (k per d_in column keeps
              |values| <= 15.5; k==0 for this data)
      - vs  = sign(V) * 2^k[d]        e3m4, EXACT (+-pow2)
      - us  = sign(U).T               e3m4, EXACT (+-1)   (r, d_out)
      - ell folded into y1 evac (per-partition f32 scalar on DVE)
      - h, bias folded into GEMM2 evac (per-partition f32 scale+bias)
      - y written bf16, upconverted on host.
  * Device per chunk c (512 tokens):
      warmup: ~6 matmuls on a memset zeros tile bridge the DMA-wait so the
              PE HAM clock is warm when real matmuls start.
      GEMM1: y1(r=128, 512) += vs[:,dt,:].T @ xq[:,dt,:] over 32 dt (PSUM)
      y1 -> bf16 via DVE tensor_scalar_mul by ell (f32, exact fold)
      GEMM2: out(o=128, 512) = us[:,ot].T @ y1 per ot; evac applies
             out*h + bias via ACT activation(scale,bias) or DVE
             tensor_scalar(mult,add), pattern A:V = 17:15 per chunk.
      GEMM2(c0) is interleaved 1:1 with GEMM1(c1) on the PE so the
      evac/store stream never pauses while chunk 1 accumulates.
      out groups of 8 ot DMA'd out on sync/gpsimd; final group split 3-way
      (sync/gpsimd/scalar) to shorten the drain tail.
  * Layouts fully host-packed so every DMA is contiguous per partition:
      xq  [p, c, dt, t]   chunks c of 512 tokens, dt = d_in/128 tile
      vs  [p, dt, r]
      us  [r, d_out]
      hb  [p, 2*N_OT+1]   h cols, bias cols, ell col
      y   [p, c, ot, t]
"""

import ml_dtypes
import numpy as np

import concourse.bass as bass
import concourse.mybir as mybir
import concourse.tile as tile
from concourse.bass_utils import run_bass_kernel_spmd

N_CORES = 8
B, S, D_IN, D_OUT, R = 4, 2048, 4096, 4096, 128
T = B * S                      # 8192 tokens
T_CORE = T // N_CORES          # 1024 tokens per core
T_CHUNK = 512                  # tokens per chunk (one PSUM bank of f32)
N_CHUNKS = T_CORE // T_CHUNK
P = 128
N_DT = D_IN // P               # 32 d_in tiles
N_OT = D_OUT // P              # 32 d_out tiles
O_GRP = 8                      # ot tiles per out DMA (1 MiB)
F32 = mybir.dt.float32
BF16 = mybir.dt.bfloat16
FP8 = mybir.dt.float8e3
E3M4_MAX = 15.5
N_WARM = 6                     # warmup matmuls (N=512) to bridge DMA wait

_CACHED = {}

# evac engine per ot: A=scalar ACT activation (~570ns/tile),
# V=vector DVE tensor_scalar (~658ns/tile); 17:15 balances the lanes.
_EVAC = "AV" * 15 + "AA"


def _build_nc():
    from concourse.bacc import Bacc
    nc = Bacc()
    xq = nc.dram_tensor("xq", [P, N_CHUNKS * N_DT * T_CHUNK], FP8,
                        kind="ExternalInput")
    vs = nc.dram_tensor("vs", [P, N_DT * R], FP8, kind="ExternalInput")
    us = nc.dram_tensor("us", [P, D_OUT], FP8, kind="ExternalInput")
    hb = nc.dram_tensor("hb", [P, 2 * N_OT + 1], F32, kind="ExternalInput")
    y = nc.dram_tensor("y", [P, N_CHUNKS * N_OT * T_CHUNK], BF16,
                       kind="ExternalOutput")

    with tile.TileContext(nc) as tc:
        with (
            tc.tile_pool(name="params", bufs=1) as ppool,
            tc.tile_pool(name="xin", bufs=2) as xpool,
            tc.tile_pool(name="y1sb", bufs=2) as y1pool,
            tc.tile_pool(name="outsb", bufs=2) as opool,
            tc.tile_pool(name="ps_w", bufs=1, space=bass.MemorySpace.PSUM) as psw,
            tc.tile_pool(name="ps_y1", bufs=1, space=bass.MemorySpace.PSUM) as ps1,
            tc.tile_pool(name="ps_o", bufs=5, space=bass.MemorySpace.PSUM) as ps2,
        ):
            # ---- warmup zeros (gpsimd memset is its first op, ~free) ----
            zx = ppool.tile([P, T_CHUNK], FP8)
            nc.gpsimd.memset(zx[:], 0)

            # ---- params: vs whole on sync (needed first), hb on scalar,
            # us on gpsimd mid-stream (needed only by GEMM2 at ~16us) ----
            vs_sb = ppool.tile([P, N_DT, R], FP8)
            nc.sync.dma_start(vs_sb[:],
                              vs[:].rearrange("p (n r) -> p n r", n=N_DT))
            hb_sb = ppool.tile([P, 2 * N_OT + 1], F32)
            nc.scalar.dma_start(hb_sb[:], hb[:])
            us_sb = ppool.tile([P, D_OUT], FP8)

            # ---- x ladder: small pieces first so GEMM1 starts early;
            # alternating sync/gpsimd keeps arrival roughly in dt order ----
            x_sb = [xpool.tile([P, N_DT * T_CHUNK], FP8, tag="x", name=f"x{c}")
                    for c in range(N_CHUNKS)]

            def xdma(c, dt0, dt1, q):
                lo = c * N_DT * T_CHUNK + dt0 * T_CHUNK
                hi = c * N_DT * T_CHUNK + dt1 * T_CHUNK
                q.dma_start(x_sb[c][:, dt0 * T_CHUNK:dt1 * T_CHUNK],
                            xq[:, lo:hi])

            xdma(0, 0, 2, nc.sync)
            xdma(0, 2, 4, nc.gpsimd)
            xdma(0, 4, 8, nc.sync)
            xdma(0, 8, 12, nc.gpsimd)
            xdma(0, 12, 20, nc.sync)
            xdma(0, 20, 26, nc.gpsimd)
            xdma(0, 26, 32, nc.sync)
            nc.gpsimd.dma_start(us_sb[:], us[:])
            xdma(1, 0, 8, nc.sync)
            xdma(1, 8, 16, nc.gpsimd)
            xdma(1, 16, 24, nc.sync)
            xdma(1, 24, 32, nc.gpsimd)

            # ---- PSUM/out tiles ----
            wps = psw.tile([P, T_CHUNK], F32)
            g1ps = [ps1.tile([R, T_CHUNK], F32, name=f"y1ps{c}")
                    for c in range(N_CHUNKS)]
            y1_sb = [y1pool.tile([R, T_CHUNK], BF16, name=f"y1sb{c}")
                     for c in range(N_CHUNKS)]
            out_sb = [opool.tile([P, N_OT * T_CHUNK], BF16, name=f"osb{c}")
                      for c in range(N_CHUNKS)]

            ell_ap = hb_sb[:, 2 * N_OT:2 * N_OT + 1]

            # ---- warmup matmuls: keep PE busy from the barrier so HAM is
            # at full clock when real matmuls start ----
            for _ in range(N_WARM):
                nc.tensor.matmul(wps[:], zx[:, 0:P], zx[:],
                                 start=True, stop=True)

            def g1_mm(c, dt):
                nc.tensor.matmul(
                    g1ps[c][:],
                    vs_sb[:, dt, :],
                    x_sb[c][:, dt * T_CHUNK:(dt + 1) * T_CHUNK],
                    start=(dt == 0),
                    stop=(dt == N_DT - 1),
                )

            def y1_evac(c):
                nc.vector.tensor_scalar_mul(y1_sb[c][:], g1ps[c][:], ell_ap)

            def g2_step(c, ot):
                ps = ps2.tile([P, T_CHUNK], F32)
                nc.tensor.matmul(ps[:], us_sb[:, ot * P:(ot + 1) * P],
                                 y1_sb[c][:], start=True, stop=True)
                osl = out_sb[c][:, ot * T_CHUNK:(ot + 1) * T_CHUNK]
                h_ap = hb_sb[:, ot:ot + 1]
                b_ap = hb_sb[:, N_OT + ot:N_OT + ot + 1]
                if _EVAC[ot] == "A":
                    nc.scalar.activation(
                        osl, ps[:], mybir.ActivationFunctionType.Identity,
                        bias=b_ap, scale=h_ap)
                else:
                    nc.vector.tensor_scalar(
                        osl, ps[:], h_ap, b_ap,
                        mybir.AluOpType.mult, mybir.AluOpType.add)
                if ot % O_GRP == O_GRP - 1:
                    g0 = ot - (O_GRP - 1)
                    gidx = c * (N_OT // O_GRP) + ot // O_GRP
                    if gidx == N_CHUNKS * (N_OT // O_GRP) - 1:
                        # final group split 3 ways to halve the drain tail
                        for h0, h1, dq in ((g0, g0 + 4, nc.sync),
                                           (g0 + 4, g0 + 6, nc.gpsimd),
                                           (g0 + 6, g0 + 8, nc.scalar)):
                            lo = c * N_OT * T_CHUNK + h0 * T_CHUNK
                            dq.dma_start(
                                y[:, lo:lo + (h1 - h0) * T_CHUNK],
                                out_sb[c][:, h0 * T_CHUNK:h1 * T_CHUNK])
                    else:
                        lo = c * N_OT * T_CHUNK + g0 * T_CHUNK
                        dq = (nc.sync, nc.gpsimd)[gidx % 2]
                        dq.dma_start(
                            y[:, lo:lo + O_GRP * T_CHUNK],
                            out_sb[c][:, g0 * T_CHUNK:(ot + 1) * T_CHUNK])

            # ---- PE stream: G1c0; then G2c0 interleaved 1:1 with G1c1 so
            # the evac/store stream never pauses; then G2c1 ----
            for dt in range(N_DT):
                g1_mm(0, dt)
            y1_evac(0)
            for k in range(N_OT):
                g2_step(0, k)
                g1_mm(1, k)
            y1_evac(1)
            for k in range(N_OT):
                g2_step(1, k)

    nc.finalize()
    return nc


def _get_nc():
    if "nc" not in _CACHED:
        _CACHED["nc"] = _build_nc()
    return _CACHED["nc"]


def _prep_inputs(x, U_fp, V_fp, h, g, ell, bias):
    x = np.asarray(x, dtype=np.float32).reshape(T, D_IN)
    U_fp = np.asarray(U_fp, dtype=np.float32)
    V_fp = np.asarray(V_fp, dtype=np.float32)
    h = np.asarray(h, dtype=np.float32)
    g = np.asarray(g, dtype=np.float32)
    ell = np.asarray(ell, dtype=np.float32)
    bias = np.asarray(bias, dtype=np.float32)

    U_sign = np.where(U_fp >= 0, np.float32(1.0), np.float32(-1.0))
    V_sign = np.where(V_fp >= 0, np.float32(1.0), np.float32(-1.0))

    np_fp8 = mybir.dt.np(FP8)
    xg = x * g[None, :]
    # per-column power-of-two scale so |xq| <= 15.5 (exact inverse on vs)
    mx = np.abs(xg).max(axis=0)
    k = np.maximum(0, np.ceil(np.log2(np.maximum(mx, 1e-30) / E3M4_MAX)))
    k = k.astype(np.float32)
    assert k.max() <= 3.0, "pow2 scale exceeds e3m4 range"
    scale = (2.0 ** k).astype(np.float32)
    xh = np.clip(xg / scale[None, :], -E3M4_MAX, E3M4_MAX).astype(np_fp8)
    vs_host = (V_sign * scale[:, None]).astype(np_fp8)

    # pack vs (d_in, r) -> (p, dt*r)
    vs_host = np.ascontiguousarray(
        vs_host.reshape(N_DT, P, R).transpose(1, 0, 2).reshape(P, N_DT * R))
    us_host = np.ascontiguousarray(U_sign.T.astype(np_fp8))       # (r, d_out)
    hb_host = np.zeros((P, 2 * N_OT + 1), np.float32)
    hb_host[:, :N_OT] = h.reshape(N_OT, P).T
    hb_host[:, N_OT:2 * N_OT] = bias.reshape(N_OT, P).T
    hb_host[:, 2 * N_OT] = ell

    in_maps = []
    for cidx in range(N_CORES):
        shard = xh[cidx * T_CORE:(cidx + 1) * T_CORE]      # (1024, 4096)
        xp = shard.reshape(N_CHUNKS, T_CHUNK, N_DT, P)
        xp = np.ascontiguousarray(
            xp.transpose(3, 0, 2, 1).reshape(P, N_CHUNKS * N_DT * T_CHUNK))
        in_maps.append({
            "xq": xp,
            "vs": vs_host,
            "us": us_host,
            "hb": hb_host,
        })
    return in_maps


def _unpack_core(yp):
    """(P, N_CHUNKS*N_OT*T_CHUNK) packed bf16 -> (T_CORE, D_OUT) f32."""
    yp = np.asarray(yp).reshape(P, N_CHUNKS, N_OT, T_CHUNK)
    return yp.transpose(1, 3, 2, 0).reshape(T_CORE, D_OUT).astype(np.float32)


def _unpack_output(res):
    outs = [_unpack_core(res.results[c]["y"]) for c in range(N_CORES)]
    return np.concatenate(outs, axis=0).reshape(B, S, D_OUT)


def kernel(x, U_fp, V_fp, h, g, ell, bias, _run_kwargs=None):
    in_maps = _prep_inputs(x, U_fp, V_fp, h, g, ell, bias)
    nc = _get_nc()
    kw = _run_kwargs or {}
    res = run_bass_kernel_spmd(nc, in_maps, list(range(N_CORES)), **kw)
    if _run_kwargs is not None:
        _CACHED["last_results"] = res
    return _unpack_output(res)
